# revision 27
# baseline (speedup 1.0000x reference)
"""Trainium2 Bass kernel for nn_BaseQuantizer (multi-scale VQ codebook).

Strategy (8 cores, data-parallel over batch B=16 -> 2 images/core, no
collectives):
  per level pn in (1,2,4,8,16,32):
    zd      = pool(z_enc) - pool(z_dec)          (linearity: pool(z_rest))
    scores  = zd^T @ embT - esq/2                (PE, all-fp32r)
    token   = argmax over V=8192                 (DVE Max8 + MaxIndex)
    zq      = emb[token]                         (gpsimd indirect DMA gather)
    zu      = W_l-interp matmul / transpose      (PE, fp32r)
    z_dec  += zu ; write level output            (DVE + DMA)

Layouts: activations kept as [C=128 partitions, tokens] so the score GEMM
uses token tiles as the stationary operand and embT [128, 8192] as the
moving operand. Gathered codebook rows land as [token partition, C], which
is exactly the stationary operand the separable-bicubic interp matmul needs
(W_l = kron(Mh, Mw) precomputed on host); its output is [C, H*W] — the
z_dec layout.  Levels with pn<=8 pack both images into one padded token
tile (image 0 at partition/col 0, image 1 at 64).

Hardware notes baked into this design (all measured on TRN2 here):
 - Every matmul is fp32r with K=128 stationary: mixing dtypes (bf16/f32r)
   or stationary K sizes (3 vs 128) between back-to-back matmuls keeps the
   PE at its cold 1.2 GHz clock (630+ ns per N=512 matmul vs 227 ns warm).
   Hence the bias rows are padded into a K=128 matmul.
 - fp32r hardware precision is ~11-bit mantissa on the inputs; top-2
   argmin gaps (p50=2.7, p1=0.04) make flips rare, and even an exact-fp32
   reimplementation differs from the jax reference by ~1.4e-2 output
   rel-err through argmin flips (measured), so ~1.7e-2 is at the floor.
 - The DVE Max8/FindIndex8 full-row scans (1 elem/cycle, no fast mode for
   any dtype) are the throughput wall: 2 passes x 8192 x 24 tiles.
"""
import numpy as np
import ml_dtypes

# ---------------------------------------------------------------- constants
MS = (1, 2, 4, 8, 16, 32)
A_COEF = -0.75
B_FULL = 16
B_LOC = 2          # images per core
N_CORES = 8
C = 128
V = 8192
H = W_DIM = 32
HW = 1024
VBLK = 512
NBLK = V // VBLK   # 16


def _cubic_w(t, a=A_COEF):
    t = abs(float(t))
    if t <= 1.0:
        return (a + 2.0) * t**3 - (a + 3.0) * t**2 + 1.0
    if t < 2.0:
        return a * t**3 - 5.0 * a * t**2 + 8.0 * a * t - 4.0 * a
    return 0.0


def _bicubic_matrix(n_in, n_out):
    M = np.zeros((n_out, n_in), np.float32)
    s = n_in / n_out
    for i in range(n_out):
        x = (i + 0.5) * s - 0.5
        x0 = int(np.floor(x))
        t = x - x0
        for off in (-1, 0, 1, 2):
            j = min(max(x0 + off, 0), n_in - 1)
            M[i, j] += _cubic_w(off - t)
    return M


def _to_bf16(x):
    return x.astype(ml_dtypes.bfloat16)


# ---------------------------------------------------------------- builder
_CACHE = {}


def _build():
    import concourse.bacc as bacc
    import concourse.tile as tile
    from concourse import mybir
    import concourse.bass as bass
    from concourse.bass import IndirectOffsetOnAxis

    f32 = mybir.dt.float32
    f32r = mybir.dt.float32r
    bf16 = mybir.dt.bfloat16
    u32 = mybir.dt.uint32

    nc = bacc.Bacc("TRN2", target_bir_lowering=False, debug=False,
                   num_devices=N_CORES)

    # ------------- dram io
    # Constants consumed by fp32r matmuls are declared float32r in DRAM so a
    # plain DMA satisfies the compiler's "rounded to FP32r" producer check
    # (hardware rounds internally; numpy side is plain float32 bits).
    # Everything on the PE is fp32r: mixing bf16 and fp32r matmuls keeps the
    # PE at its cold clock (measured 630 vs 227 ns per N=512 matmul).
    z_in = nc.dram_tensor("z", [B_LOC, C, HW], f32, kind="ExternalInput")
    emb_d = nc.dram_tensor("emb", [V, C], f32r, kind="ExternalInput")
    embT_d = nc.dram_tensor("embT", [C, V], f32r, kind="ExternalInput")
    bias3_d = nc.dram_tensor("bias3", [C, V], f32r, kind="ExternalInput")
    ones3_d = nc.dram_tensor("ones3", [C, C], f32r, kind="ExternalInput")
    i128_d = nc.dram_tensor("i128", [C, C], f32r, kind="ExternalInput")
    w_d = []
    for l in range(5):
        pn = MS[l]
        if pn <= 8:
            shp = [C, HW]            # dual-band padded
        else:
            shp = [C, (pn * pn // C) * HW]   # chunk-packed [128, nchunk*1024]
        w_d.append(nc.dram_tensor(f"w{l}", shp, f32r, kind="ExternalInput"))
    pe_d = []
    for l in range(5):
        pn = MS[l]
        cols = C if pn <= 8 else B_LOC * pn * pn
        pe_d.append(nc.dram_tensor(f"pe{l}", [C, cols], f32,
                                   kind="ExternalInput"))
    out_d = nc.dram_tensor("out", [len(MS), B_LOC, C, HW], f32,
                           kind="ExternalOutput")

    with tile.TileContext(nc) as tc:
        with (
            tc.tile_pool(name="const", bufs=1) as cpool,
            tc.tile_pool(name="zdec", bufs=1) as zdec_pool,
            tc.tile_pool(name="zd", bufs=2) as zd_pool,
            tc.tile_pool(name="scores", bufs=2) as sc_pool,
            tc.tile_pool(name="small", bufs=4) as small_pool,
            tc.tile_pool(name="zq", bufs=12) as zq_pool,
            tc.tile_pool(name="ptmp", bufs=2) as ptmp_pool,
            tc.tile_pool(name="spsum", bufs=3, space="PSUM") as spsum_pool,
            tc.tile_pool(name="upsum", bufs=1, space="PSUM") as upsum_pool,
        ):
            # ---------------- load constants
            # pe0 + embT + bias3 gate level-0's first matmuls: load them
            # first, split across DMA queues; everything else after.
            pe_sb = []
            pe0_t = cpool.tile(list(pe_d[0].shape), f32, tag="pe0")
            nc.sync.dma_start(pe0_t[:], pe_d[0][:])
            pe_sb.append(pe0_t)

            # bias matmul stays K=128 (stationary-K alternation with the
            # K=128 score matmuls would keep the PE at its slow clock):
            # rows 0-2 of bias3 hold the bf16-style 3-way split, rows 3+
            # are zero; ones3 selects/sums them for every token column.
            # embT/bias3 live as 4 chunk tiles each so the first matmuls
            # depend on 2MB of DMA, not 8MB (tile-granular deps).
            embT_c, bias3_c = [], []
            for q in range(4):
                et = cpool.tile([C, V // 4], f32r, tag=f"embT{q}",
                                name=f"embT{q}")
                nc.sync.dma_start(et[:], embT_d[:, 2048 * q: 2048 * (q + 1)])
                embT_c.append(et)
                bt = cpool.tile([C, V // 4], f32r, tag=f"bias3{q}",
                                name=f"bias3{q}")
                nc.sync.dma_start(bt[:], bias3_d[:, 2048 * q: 2048 * (q + 1)])
                bias3_c.append(bt)
            ones3 = cpool.tile([C, C], f32r)
            nc.sync.dma_start(ones3[:], ones3_d[:])

            for l in range(1, 5):
                t = cpool.tile(list(pe_d[l].shape), f32, tag=f"pe{l}")
                nc.sync.dma_start(t[:], pe_d[l][:])
                pe_sb.append(t)

            z_sb = cpool.tile([C, B_LOC, HW], f32)
            nc.sync.dma_start(
                z_sb[:], z_in[:].rearrange("b c x -> c b x"))

            i128 = cpool.tile([C, C], f32r)
            nc.sync.dma_start(i128[:], i128_d[:])

            w_sb = []
            for l in range(5):
                shp = list(w_d[l].shape)
                wt = cpool.tile(shp, f32r, tag=f"w{l}")
                nc.sync.dma_start(wt[:], w_d[l][:])
                w_sb.append(wt)

            z_dec = zdec_pool.tile([C, B_LOC, HW], f32, tag="zdec")
            nc.vector.memset(z_dec[:], 0.0)

            # ---------------- helpers
            def emit_pool(b, pn):
                """Sum-pool z_dec[:, b] (32x32) down to pn x pn in a single
                reduce: view as [c, ph, pw, fh, fw] and reduce the two
                innermost (XY) axes. Returns the tile with [C, pn*pn] sums."""
                f = H // pn
                t1 = ptmp_pool.tile([C, pn * pn], f32, tag="pt1")
                nc.vector.tensor_reduce(
                    t1[:],
                    z_dec[:, b].rearrange(
                        "c (ph fh pw fw) -> c ph pw fh fw",
                        ph=pn, fh=f, pw=pn, fw=f),
                    axis=mybir.AxisListType.XY,
                    op=mybir.AluOpType.add)
                return t1

            def emit_score_tile(tok_ap):
                """Score GEMM + argmax + gather for one <=128-token tile
                (tok_ap: [C, <=128] fp32r stationary). Returns zq tile."""
                scores = sc_pool.tile([C, V], f32, tag="scores")
                for pb in range(NBLK // 2):
                    ps = spsum_pool.tile([C, 2 * VBLK], f32, tag="spsum")
                    for hh in range(2):
                        blk = 2 * pb + hh
                        q, off = blk // 4, (blk % 4) * VBLK
                        nc.tensor.matmul(
                            ps[:, VBLK * hh: VBLK * (hh + 1)],
                            ones3[:],
                            bias3_c[q][:, off: off + VBLK],
                            start=True, stop=False)
                        nc.tensor.matmul(
                            ps[:, VBLK * hh: VBLK * (hh + 1)],
                            tok_ap,
                            embT_c[q][:, off: off + VBLK],
                            start=False, stop=True)
                    nc.scalar.copy(
                        scores[:, 2 * VBLK * pb: 2 * VBLK * (pb + 1)], ps[:])
                # quarter-split max so the scan starts right after the first
                # two copies instead of after half of them
                mq = []
                for qq in range(4):
                    m8q = small_pool.tile([C, 8], f32, tag=f"m8q{qq}",
                                          name=f"m8q{qq}")
                    nc.vector.max(m8q[:], scores[:, (V // 4) * qq:
                                                  (V // 4) * (qq + 1)])
                    mq.append(m8q)
                m8e = small_pool.tile([C, 8], f32, tag="m8e")
                m8f = small_pool.tile([C, 8], f32, tag="m8f")
                m8c = small_pool.tile([C, 8], f32, tag="m8c")
                i8 = small_pool.tile([C, 8], u32, tag="i8")
                nc.vector.tensor_tensor(m8e[:], mq[0][:], mq[1][:],
                                        op=mybir.AluOpType.max)
                nc.vector.tensor_tensor(m8f[:], mq[2][:], mq[3][:],
                                        op=mybir.AluOpType.max)
                nc.vector.tensor_tensor(m8c[:], m8e[:], m8f[:],
                                        op=mybir.AluOpType.max)
                nc.vector.max_index(i8[:], m8c[:], scores[:])
                zq = zq_pool.tile([C, C], f32r, tag="zq")
                nc.gpsimd.indirect_dma_start(
                    zq[:], None,
                    emb_d[:, :],
                    IndirectOffsetOnAxis(ap=i8[:, 0:1], axis=0))
                return zq

            def emit_batch_tail(l, b, zq_list, npix, last, pn):
                """Interp/transpose zq -> zu, accumulate z_dec, write out."""
                zu = upsum_pool.tile([C, HW], f32, tag="zu")
                if last:
                    for k in range(HW // C):
                        nc.tensor.matmul(
                            zu[:, C * k: C * (k + 1)],
                            zq_list[k][:], i128[:], start=True, stop=True)
                elif pn <= 8:
                    zq = zq_list[0]
                    for ns in range(2):
                        nc.tensor.matmul(
                            zu[:, VBLK * ns: VBLK * (ns + 1)],
                            zq[64 * b: 64 * b + npix, :],
                            w_sb[l][64 * b: 64 * b + npix,
                                    VBLK * ns: VBLK * (ns + 1)],
                            start=True, stop=True)
                else:
                    nchunk = npix // C
                    for ns in range(2):
                        for ci in range(nchunk):
                            nc.tensor.matmul(
                                zu[:, VBLK * ns: VBLK * (ns + 1)],
                                zq_list[ci][:],
                                w_sb[l][:, HW * ci + VBLK * ns:
                                        HW * ci + VBLK * (ns + 1)],
                                start=(ci == 0), stop=(ci == nchunk - 1))
                nc.vector.tensor_add(z_dec[:, b], z_dec[:, b], zu[:])
                nc.sync.dma_start(out_d[l, b], z_dec[:, b])

            # ---------------- per level
            for l, pn in enumerate(MS):
                last = l == len(MS) - 1
                npix = pn * pn

                if pn <= 8:
                    # both images in one padded token tile (cols 0 / 64)
                    zd = zd_pool.tile([C, C], f32r, tag="zd")
                    if l == 0:
                        nc.vector.tensor_copy(zd[:], pe_sb[0][:])
                    else:
                        f = H // pn
                        nc.vector.tensor_copy(zd[:], pe_sb[l][:])
                        for b in range(B_LOC):
                            pool_t = emit_pool(b, pn)
                            nc.vector.scalar_tensor_tensor(
                                zd[:, 64 * b: 64 * b + npix],
                                pool_t[:], -1.0 / (f * f),
                                pe_sb[l][:, 64 * b: 64 * b + npix],
                                op0=mybir.AluOpType.mult,
                                op1=mybir.AluOpType.add)
                    zq = emit_score_tile(zd[:, 0:C])
                    for b in range(B_LOC):
                        emit_batch_tail(l, b, [zq], npix, last, pn)
                else:
                    # independent per-image chains (pipeline across images);
                    # tails emitted after both chains so the DVE never idles
                    # waiting on a tail's gather->interp->add latency.
                    zq_lists = []
                    for b in range(B_LOC):
                        zd = zd_pool.tile([C, npix], f32r, tag="zd")
                        if last:
                            nc.vector.tensor_sub(
                                zd[:], z_sb[:, b], z_dec[:, b])
                        else:
                            f = H // pn
                            pool_t = emit_pool(b, pn)
                            nc.vector.scalar_tensor_tensor(
                                zd[:], pool_t[:], -1.0 / (f * f),
                                pe_sb[l][:, npix * b: npix * (b + 1)],
                                op0=mybir.AluOpType.mult,
                                op1=mybir.AluOpType.add)
                        zq_list = []
                        for tt in range(npix // C):
                            zq_list.append(
                                emit_score_tile(zd[:, C * tt: C * (tt + 1)]))
                        zq_lists.append(zq_list)
                    for b in range(B_LOC):
                        emit_batch_tail(l, b, zq_lists[b], npix, last, pn)

    nc.compile()
    return nc


def _host_inputs(z_enc, emb):
    """Build the per-core input maps."""
    z_enc = np.ascontiguousarray(z_enc, dtype=np.float32)
    emb = np.ascontiguousarray(emb, dtype=np.float32)

    embT = np.ascontiguousarray(emb.T)
    e_sq = np.sum(emb * emb, axis=1)
    bias = (-0.5 * e_sq).astype(np.float32)
    # Mean-center (a constant shift over v never changes the argmax) so the
    # bf16-style 3-way split has small residual, then store the split values
    # as f32 bits in a float32r tensor: each piece is an 8-bit-mantissa
    # number, exactly representable after any hardware fp32r rounding.
    bias = bias - np.float32(bias.mean())
    b1 = _to_bf16(bias).astype(np.float32)
    b2 = _to_bf16(bias - b1).astype(np.float32)
    b3 = _to_bf16(bias - b1 - b2).astype(np.float32)
    bias3 = np.zeros((C, V), np.float32)
    bias3[0], bias3[1], bias3[2] = b1, b2, b3
    ones3 = np.zeros((C, C), np.float32)
    ones3[0:3, :] = 1.0
    i128 = np.eye(C, dtype=np.float32)

    w_mats = []
    for l in range(5):
        pn = MS[l]
        Mh = _bicubic_matrix(pn, H)
        Mw = _bicubic_matrix(pn, W_DIM)
        Wl = np.einsum('hi,wj->ijhw', Mh, Mw).reshape(pn * pn, HW)
        Wl = Wl.astype(np.float32)
        if pn <= 8:
            wp = np.zeros((C, HW), np.float32)
            wp[0:pn * pn] = Wl
            wp[64:64 + pn * pn] = Wl
        else:
            nchunk = pn * pn // C
            wp = np.zeros((C, nchunk * HW), np.float32)
            for ci in range(nchunk):
                wp[:, HW * ci: HW * (ci + 1)] = Wl[C * ci: C * (ci + 1)]
        w_mats.append(wp)

    in_maps = []
    for c in range(N_CORES):
        zc = z_enc[B_LOC * c: B_LOC * (c + 1)]          # [2,128,32,32]
        m = {
            "z": np.ascontiguousarray(zc.reshape(B_LOC, C, HW)),
            "emb": emb,
            "embT": embT,
            "bias3": bias3,
            "ones3": ones3,
            "i128": i128,
        }
        for l in range(5):
            m[f"w{l}"] = w_mats[l]
            pn = MS[l]
            f = H // pn
            pooled = zc.reshape(B_LOC, C, pn, f, pn, f).mean(axis=(3, 5))
            pooled = pooled.reshape(B_LOC, C, pn * pn)   # [b, c, npix]
            if pn <= 8:
                pel = np.zeros((C, C), np.float32)
                for b in range(B_LOC):
                    pel[:, 64 * b: 64 * b + pn * pn] = pooled[b]
            else:
                pel = np.concatenate([pooled[0], pooled[1]], axis=1)
            m[f"pe{l}"] = np.ascontiguousarray(pel)
        in_maps.append(m)
    return in_maps


def _run(z_enc, emb, trace=False):
    from concourse.bass_utils import run_bass_kernel_spmd

    if "nc" not in _CACHE:
        _CACHE["nc"] = _build()
    nc = _CACHE["nc"]
    in_maps = _host_inputs(z_enc, emb)
    res = run_bass_kernel_spmd(nc, in_maps, core_ids=list(range(N_CORES)),
                               trace=trace)
    outs = []
    for c in range(N_CORES):
        o = res.results[c]["out"]                        # [6, 2, 128, 1024]
        outs.append(o.reshape(len(MS), B_LOC, C, H, W_DIM))
    full = np.concatenate(outs, axis=1)                  # [6, 16, 128, 32, 32]
    return full, res


def kernel(z_enc, emb):
    out, _ = _run(z_enc, emb, trace=False)
    return out


# revision 28
# speedup vs baseline: 1.2027x; 1.2027x over previous
"""Trainium2 Bass kernel for nn_BaseQuantizer (multi-scale VQ codebook).

Strategy (8 cores, data-parallel over batch B=16 -> 2 images/core, no
collectives):
  per level pn in (1,2,4,8,16,32):
    zd      = pool(z_enc) - pool(z_dec)          (linearity: pool(z_rest))
    scores  = zd^T @ embT - esq/2                (PE, all-fp32r)
    token   = argmax over V=8192                 (DVE Max8 + MaxIndex)
    zq      = emb[token]                         (gpsimd indirect DMA gather)
    zu      = W_l-interp matmul / transpose      (PE, fp32r)
    z_dec  += zu ; write level output            (DVE + DMA)

Layouts: activations kept as [C=128 partitions, tokens] so the score GEMM
uses token tiles as the stationary operand and embT [128, 8192] as the
moving operand. Gathered codebook rows land as [token partition, C], which
is exactly the stationary operand the separable-bicubic interp matmul needs
(W_l = kron(Mh, Mw) precomputed on host); its output is [C, H*W] — the
z_dec layout.  Levels with pn<=8 pack both images into one padded token
tile (image 0 at partition/col 0, image 1 at 64).

Hardware notes baked into this design (all measured on TRN2 here):
 - Every matmul is fp32r with K=128 stationary: mixing dtypes (bf16/f32r)
   or stationary K sizes (3 vs 128) between back-to-back matmuls keeps the
   PE at its cold 1.2 GHz clock (630+ ns per N=512 matmul vs 227 ns warm).
   Hence the bias rows are padded into a K=128 matmul.
 - fp32r hardware precision is ~11-bit mantissa on the inputs; top-2
   argmin gaps (p50=2.7, p1=0.04) make flips rare, and even an exact-fp32
   reimplementation differs from the jax reference by ~1.4e-2 output
   rel-err through argmin flips (measured), so ~1.7e-2 is at the floor.
 - The DVE Max8/FindIndex8 full-row scans (1 elem/cycle, no fast mode for
   any dtype) are the throughput wall: 2 passes x 8192 x 24 tiles.
"""
import numpy as np
import ml_dtypes

# ---------------------------------------------------------------- constants
MS = (1, 2, 4, 8, 16, 32)
A_COEF = -0.75
B_FULL = 16
B_LOC = 2          # images per core
N_CORES = 8
C = 128
V = 8192
H = W_DIM = 32
HW = 1024
VBLK = 512
NBLK = V // VBLK   # 16


def _cubic_w(t, a=A_COEF):
    t = abs(float(t))
    if t <= 1.0:
        return (a + 2.0) * t**3 - (a + 3.0) * t**2 + 1.0
    if t < 2.0:
        return a * t**3 - 5.0 * a * t**2 + 8.0 * a * t - 4.0 * a
    return 0.0


def _bicubic_matrix(n_in, n_out):
    M = np.zeros((n_out, n_in), np.float32)
    s = n_in / n_out
    for i in range(n_out):
        x = (i + 0.5) * s - 0.5
        x0 = int(np.floor(x))
        t = x - x0
        for off in (-1, 0, 1, 2):
            j = min(max(x0 + off, 0), n_in - 1)
            M[i, j] += _cubic_w(off - t)
    return M


def _to_bf16(x):
    return x.astype(ml_dtypes.bfloat16)


# ---------------------------------------------------------------- builder
_CACHE = {}


def _build():
    import concourse.bacc as bacc
    import concourse.tile as tile
    from concourse import mybir
    import concourse.bass as bass
    from concourse.bass import IndirectOffsetOnAxis

    f32 = mybir.dt.float32
    f32r = mybir.dt.float32r
    bf16 = mybir.dt.bfloat16
    u32 = mybir.dt.uint32

    nc = bacc.Bacc("TRN2", target_bir_lowering=False, debug=False,
                   num_devices=N_CORES)

    # ------------- dram io
    # Constants consumed by fp32r matmuls are declared float32r in DRAM so a
    # plain DMA satisfies the compiler's "rounded to FP32r" producer check
    # (hardware rounds internally; numpy side is plain float32 bits).
    # Everything on the PE is fp32r: mixing bf16 and fp32r matmuls keeps the
    # PE at its cold clock (measured 630 vs 227 ns per N=512 matmul).
    z_in = nc.dram_tensor("z", [B_LOC, C, HW], f32, kind="ExternalInput")
    emb_d = nc.dram_tensor("emb", [V, C], f32r, kind="ExternalInput")
    embT_d = nc.dram_tensor("embT", [C, V], f32r, kind="ExternalInput")
    bias3_d = nc.dram_tensor("bias3", [C, V], f32r, kind="ExternalInput")
    ones3_d = nc.dram_tensor("ones3", [C, C], f32r, kind="ExternalInput")
    i128_d = nc.dram_tensor("i128", [C, C], f32r, kind="ExternalInput")
    w_d = []
    for l in range(5):
        pn = MS[l]
        if pn <= 8:
            shp = [C, HW]            # dual-band padded
        else:
            shp = [C, (pn * pn // C) * HW]   # chunk-packed [128, nchunk*1024]
        w_d.append(nc.dram_tensor(f"w{l}", shp, f32r, kind="ExternalInput"))
    pe_d = []
    for l in range(5):
        pn = MS[l]
        cols = C if pn <= 8 else B_LOC * pn * pn
        pe_d.append(nc.dram_tensor(f"pe{l}", [C, cols], f32,
                                   kind="ExternalInput"))
    out_d = nc.dram_tensor("out", [len(MS), B_LOC, C, HW], f32,
                           kind="ExternalOutput")

    with tile.TileContext(nc) as tc:
        with (
            tc.tile_pool(name="const", bufs=1) as cpool,
            tc.tile_pool(name="zdec", bufs=1) as zdec_pool,
            tc.tile_pool(name="zd", bufs=2) as zd_pool,
            tc.tile_pool(name="scores", bufs=2) as sc_pool,
            tc.tile_pool(name="small", bufs=4) as small_pool,
            tc.tile_pool(name="zq", bufs=12) as zq_pool,
            tc.tile_pool(name="ptmp", bufs=2) as ptmp_pool,
            tc.tile_pool(name="spsum", bufs=3, space="PSUM") as spsum_pool,
            tc.tile_pool(name="upsum", bufs=1, space="PSUM") as upsum_pool,
        ):
            # ---------------- load constants
            # pe0 + embT + bias3 gate level-0's first matmuls: load them
            # first, split across DMA queues; everything else after.
            pe_sb = []
            pe0_t = cpool.tile(list(pe_d[0].shape), f32, tag="pe0")
            nc.sync.dma_start(pe0_t[:], pe_d[0][:])
            pe_sb.append(pe0_t)

            # bias matmul stays K=128 (stationary-K alternation with the
            # K=128 score matmuls would keep the PE at its slow clock):
            # rows 0-2 of bias3 hold the bf16-style 3-way split, rows 3+
            # are zero; ones3 selects/sums them for every token column.
            # embT/bias3 live as 4 chunk tiles each so the first matmuls
            # depend on 2MB of DMA, not 8MB (tile-granular deps).
            embT_c, bias3_c = [], []
            for q in range(4):
                et = cpool.tile([C, V // 4], f32r, tag=f"embT{q}",
                                name=f"embT{q}")
                nc.sync.dma_start(et[:], embT_d[:, 2048 * q: 2048 * (q + 1)])
                embT_c.append(et)
                bt = cpool.tile([C, V // 4], f32r, tag=f"bias3{q}",
                                name=f"bias3{q}")
                nc.sync.dma_start(bt[:], bias3_d[:, 2048 * q: 2048 * (q + 1)])
                bias3_c.append(bt)
            ones3 = cpool.tile([C, C], f32r)
            nc.sync.dma_start(ones3[:], ones3_d[:])

            for l in range(1, 5):
                t = cpool.tile(list(pe_d[l].shape), f32, tag=f"pe{l}")
                nc.sync.dma_start(t[:], pe_d[l][:])
                pe_sb.append(t)

            z_sb = cpool.tile([C, B_LOC, HW], f32)
            nc.sync.dma_start(
                z_sb[:], z_in[:].rearrange("b c x -> c b x"))

            i128 = cpool.tile([C, C], f32r)
            nc.sync.dma_start(i128[:], i128_d[:])

            w_sb = []
            for l in range(5):
                shp = list(w_d[l].shape)
                wt = cpool.tile(shp, f32r, tag=f"w{l}")
                nc.sync.dma_start(wt[:], w_d[l][:])
                w_sb.append(wt)

            z_dec = zdec_pool.tile([C, B_LOC, HW], f32, tag="zdec")
            nc.vector.memset(z_dec[:], 0.0)

            # ---------------- helpers
            def emit_pool(b, pn):
                """Sum-pool z_dec[:, b] (32x32) down to pn x pn in a single
                reduce: view as [c, ph, pw, fh, fw] and reduce the two
                innermost (XY) axes. Returns the tile with [C, pn*pn] sums."""
                f = H // pn
                t1 = ptmp_pool.tile([C, pn * pn], f32, tag="pt1")
                nc.vector.tensor_reduce(
                    t1[:],
                    z_dec[:, b].rearrange(
                        "c (ph fh pw fw) -> c ph pw fh fw",
                        ph=pn, fh=f, pw=pn, fw=f),
                    axis=mybir.AxisListType.XY,
                    op=mybir.AluOpType.add)
                return t1

            def emit_score_tile(tok_ap, fine_split=True):
                """Score GEMM + argmax + gather for one <=128-token tile
                (tok_ap: [C, <=128] fp32r stationary). Returns zq tile."""
                scores = sc_pool.tile([C, V], f32, tag="scores")
                for pb in range(NBLK // 2):
                    ps = spsum_pool.tile([C, 2 * VBLK], f32, tag="spsum")
                    for hh in range(2):
                        blk = 2 * pb + hh
                        q, off = blk // 4, (blk % 4) * VBLK
                        nc.tensor.matmul(
                            ps[:, VBLK * hh: VBLK * (hh + 1)],
                            ones3[:],
                            bias3_c[q][:, off: off + VBLK],
                            start=True, stop=False)
                        nc.tensor.matmul(
                            ps[:, VBLK * hh: VBLK * (hh + 1)],
                            tok_ap,
                            embT_c[q][:, off: off + VBLK],
                            start=False, stop=True)
                    nc.scalar.copy(
                        scores[:, 2 * VBLK * pb: 2 * VBLK * (pb + 1)], ps[:])
                # split max so the scan starts before the last copies:
                # quarters on chain-entry tiles (DVE otherwise waiting),
                # halves in steady state (less per-op overhead)
                nsplit = 4 if fine_split else 2
                mq = []
                for qq in range(nsplit):
                    m8q = small_pool.tile([C, 8], f32, tag=f"m8q{qq}",
                                          name=f"m8q{qq}")
                    nc.vector.max(m8q[:], scores[:, (V // nsplit) * qq:
                                                  (V // nsplit) * (qq + 1)])
                    mq.append(m8q)
                m8c = small_pool.tile([C, 8], f32, tag="m8c")
                i8 = small_pool.tile([C, 8], u32, tag="i8")
                if fine_split:
                    m8e = small_pool.tile([C, 8], f32, tag="m8e")
                    m8f = small_pool.tile([C, 8], f32, tag="m8f")
                    nc.vector.tensor_tensor(m8e[:], mq[0][:], mq[1][:],
                                            op=mybir.AluOpType.max)
                    nc.vector.tensor_tensor(m8f[:], mq[2][:], mq[3][:],
                                            op=mybir.AluOpType.max)
                    nc.vector.tensor_tensor(m8c[:], m8e[:], m8f[:],
                                            op=mybir.AluOpType.max)
                else:
                    nc.vector.tensor_tensor(m8c[:], mq[0][:], mq[1][:],
                                            op=mybir.AluOpType.max)
                nc.vector.max_index(i8[:], m8c[:], scores[:])
                zq = zq_pool.tile([C, C], f32r, tag="zq")
                nc.gpsimd.indirect_dma_start(
                    zq[:], None,
                    emb_d[:, :],
                    IndirectOffsetOnAxis(ap=i8[:, 0:1], axis=0))
                return zq

            def emit_batch_tail(l, b, zq_list, npix, last, pn):
                """Interp/transpose zq -> zu, accumulate z_dec, write out."""
                zu = upsum_pool.tile([C, HW], f32, tag="zu")
                if last:
                    for k in range(HW // C):
                        nc.tensor.matmul(
                            zu[:, C * k: C * (k + 1)],
                            zq_list[k][:], i128[:], start=True, stop=True)
                elif pn <= 8:
                    zq = zq_list[0]
                    for ns in range(2):
                        nc.tensor.matmul(
                            zu[:, VBLK * ns: VBLK * (ns + 1)],
                            zq[64 * b: 64 * b + npix, :],
                            w_sb[l][64 * b: 64 * b + npix,
                                    VBLK * ns: VBLK * (ns + 1)],
                            start=True, stop=True)
                else:
                    nchunk = npix // C
                    for ns in range(2):
                        for ci in range(nchunk):
                            nc.tensor.matmul(
                                zu[:, VBLK * ns: VBLK * (ns + 1)],
                                zq_list[ci][:],
                                w_sb[l][:, HW * ci + VBLK * ns:
                                        HW * ci + VBLK * (ns + 1)],
                                start=(ci == 0), stop=(ci == nchunk - 1))
                nc.vector.tensor_add(z_dec[:, b], z_dec[:, b], zu[:])
                nc.sync.dma_start(out_d[l, b], z_dec[:, b])

            # ---------------- per level
            for l, pn in enumerate(MS):
                last = l == len(MS) - 1
                npix = pn * pn

                if pn <= 8:
                    # both images in one padded token tile (cols 0 / 64)
                    zd = zd_pool.tile([C, C], f32r, tag="zd")
                    if l == 0:
                        nc.vector.tensor_copy(zd[:], pe_sb[0][:])
                    else:
                        f = H // pn
                        nc.vector.tensor_copy(zd[:], pe_sb[l][:])
                        for b in range(B_LOC):
                            pool_t = emit_pool(b, pn)
                            nc.vector.scalar_tensor_tensor(
                                zd[:, 64 * b: 64 * b + npix],
                                pool_t[:], -1.0 / (f * f),
                                pe_sb[l][:, 64 * b: 64 * b + npix],
                                op0=mybir.AluOpType.mult,
                                op1=mybir.AluOpType.add)
                    zq = emit_score_tile(zd[:, 0:C])
                    for b in range(B_LOC):
                        emit_batch_tail(l, b, [zq], npix, last, pn)
                else:
                    # independent per-image chains (pipeline across images);
                    # tails emitted after both chains so the DVE never idles
                    # waiting on a tail's gather->interp->add latency.
                    zq_lists = []
                    for b in range(B_LOC):
                        zd = zd_pool.tile([C, npix], f32r, tag="zd")
                        if last:
                            nc.vector.tensor_sub(
                                zd[:], z_sb[:, b], z_dec[:, b])
                        else:
                            f = H // pn
                            pool_t = emit_pool(b, pn)
                            nc.vector.scalar_tensor_tensor(
                                zd[:], pool_t[:], -1.0 / (f * f),
                                pe_sb[l][:, npix * b: npix * (b + 1)],
                                op0=mybir.AluOpType.mult,
                                op1=mybir.AluOpType.add)
                        zq_list = []
                        for tt in range(npix // C):
                            zq_list.append(emit_score_tile(
                                zd[:, C * tt: C * (tt + 1)],
                                fine_split=(tt == 0 and b == 0)))
                        zq_lists.append(zq_list)
                    for b in range(B_LOC):
                        emit_batch_tail(l, b, zq_lists[b], npix, last, pn)

    nc.compile()
    return nc


def _host_inputs(z_enc, emb):
    """Build the per-core input maps."""
    z_enc = np.ascontiguousarray(z_enc, dtype=np.float32)
    emb = np.ascontiguousarray(emb, dtype=np.float32)

    embT = np.ascontiguousarray(emb.T)
    e_sq = np.sum(emb * emb, axis=1)
    bias = (-0.5 * e_sq).astype(np.float32)
    # Mean-center (a constant shift over v never changes the argmax) so the
    # bf16-style 3-way split has small residual, then store the split values
    # as f32 bits in a float32r tensor: each piece is an 8-bit-mantissa
    # number, exactly representable after any hardware fp32r rounding.
    bias = bias - np.float32(bias.mean())
    b1 = _to_bf16(bias).astype(np.float32)
    b2 = _to_bf16(bias - b1).astype(np.float32)
    b3 = _to_bf16(bias - b1 - b2).astype(np.float32)
    bias3 = np.zeros((C, V), np.float32)
    bias3[0], bias3[1], bias3[2] = b1, b2, b3
    ones3 = np.zeros((C, C), np.float32)
    ones3[0:3, :] = 1.0
    i128 = np.eye(C, dtype=np.float32)

    w_mats = []
    for l in range(5):
        pn = MS[l]
        Mh = _bicubic_matrix(pn, H)
        Mw = _bicubic_matrix(pn, W_DIM)
        Wl = np.einsum('hi,wj->ijhw', Mh, Mw).reshape(pn * pn, HW)
        Wl = Wl.astype(np.float32)
        if pn <= 8:
            wp = np.zeros((C, HW), np.float32)
            wp[0:pn * pn] = Wl
            wp[64:64 + pn * pn] = Wl
        else:
            nchunk = pn * pn // C
            wp = np.zeros((C, nchunk * HW), np.float32)
            for ci in range(nchunk):
                wp[:, HW * ci: HW * (ci + 1)] = Wl[C * ci: C * (ci + 1)]
        w_mats.append(wp)

    in_maps = []
    for c in range(N_CORES):
        zc = z_enc[B_LOC * c: B_LOC * (c + 1)]          # [2,128,32,32]
        m = {
            "z": np.ascontiguousarray(zc.reshape(B_LOC, C, HW)),
            "emb": emb,
            "embT": embT,
            "bias3": bias3,
            "ones3": ones3,
            "i128": i128,
        }
        for l in range(5):
            m[f"w{l}"] = w_mats[l]
            pn = MS[l]
            f = H // pn
            pooled = zc.reshape(B_LOC, C, pn, f, pn, f).mean(axis=(3, 5))
            pooled = pooled.reshape(B_LOC, C, pn * pn)   # [b, c, npix]
            if pn <= 8:
                pel = np.zeros((C, C), np.float32)
                for b in range(B_LOC):
                    pel[:, 64 * b: 64 * b + pn * pn] = pooled[b]
            else:
                pel = np.concatenate([pooled[0], pooled[1]], axis=1)
            m[f"pe{l}"] = np.ascontiguousarray(pel)
        in_maps.append(m)
    return in_maps


def _run(z_enc, emb, trace=False):
    from concourse.bass_utils import run_bass_kernel_spmd

    if "nc" not in _CACHE:
        _CACHE["nc"] = _build()
    nc = _CACHE["nc"]
    in_maps = _host_inputs(z_enc, emb)
    res = run_bass_kernel_spmd(nc, in_maps, core_ids=list(range(N_CORES)),
                               trace=trace)
    outs = []
    for c in range(N_CORES):
        o = res.results[c]["out"]                        # [6, 2, 128, 1024]
        outs.append(o.reshape(len(MS), B_LOC, C, H, W_DIM))
    full = np.concatenate(outs, axis=1)                  # [6, 16, 128, 32, 32]
    return full, res


def kernel(z_enc, emb):
    out, _ = _run(z_enc, emb, trace=False)
    return out
